# revision 1
# baseline (speedup 1.0000x reference)
"""Trainium2 Bass kernel for fused cross+self attention (nn_Attention_3539053052516).

Strategy (8 NeuronCores, head-parallel):
  - 16 heads -> 2 heads per core. Each core computes its 2 heads' q/k/v
    projections, full attention over 4096 keys (2048 self + 2048 cross), and a
    partial output projection over its 128 o-channels. Host sums the 8 partial
    outputs and adds the bias.
  - Inputs x,y are transposed + cast to bf16 on host (layout prep) so the
    contraction dim (C) lands on SBUF partitions; per-core weight slices are
    pre-transposed/cast on host as well.
  - Rotary pairs are de-interleaved via a host-side permutation of the weight
    rows (evens then odds), so on-device rotary works on contiguous 32-wide
    slices. The same permutation is applied to q/k norm weights; attention
    scores are invariant under a consistent hd-permutation of q and k.
  - RMS-norm statistics and rotary are computed in natural [token, channel]
    layout with wide batched DVE ops; q/k are then PE-transposed into
    [hd, token] stacks for the attention matmuls.
  - Softmax: no max-subtraction (|scores*0.125| <= ~8, exp is safe in fp32);
    denominator comes from an extra ones-column appended to v (M=65 matmul);
    normalization multiplies oT by broadcast reciprocal row sums.
"""

import numpy as np
import ml_dtypes

import concourse.bass as bass
import concourse.tile as tile
from concourse import bacc, mybir
from concourse.masks import make_identity
from concourse.bass_utils import run_bass_kernel_spmd

F32 = mybir.dt.float32
BF16 = mybir.dt.bfloat16
AF = mybir.ActivationFunctionType

H = 16
HD = 64
C = 1024
NCORES = 8
HPC = H // NCORES  # heads per core = 2
EPS = 1e-6
SCALE = HD ** -0.5
DEBUG = False  # when True, dump intermediates as extra outputs
RECIP_MODE = "native"   # "approx" (custom DVE) | "native"
ATTN_BATCH = 2          # k-chunks per exp batch
PO_OWN = False          # out-proj psum: own tag (needs ATTN_BATCH<=2) or share o-banks
O_BUFS = 1              # psum bufs for o accumulators (2 needs ATTN_BATCH<=2)
BCAST_MODE = "gpsimd"   # "gpsimd" | "pe" 


def build_nc(n_tok=2048, m_tok=2048, num_devices=NCORES):
    """Build the per-core Bass program (SPMD; all cores identical)."""
    TTX = n_tok // 128   # x token tiles
    TTY = m_tok // 128   # y token tiles
    KC = TTX + TTY       # k chunks of 128 (x tokens then y tokens)
    QB = max(1, n_tok // 512)  # q blocks of 512
    QW = n_tok // QB     # q block width (512)
    XCH = 3 * HPC * HD   # x-proj output channels = 384
    YCH = 2 * HPC * HD   # y-proj output channels = 256
    KCk = C // 128       # contraction chunks = 8

    nc = bacc.Bacc("TRN2", target_bir_lowering=False, debug=False,
                   num_devices=num_devices)

    xT = nc.dram_tensor("xT", [C, n_tok], BF16, kind="ExternalInput").ap()
    yT = nc.dram_tensor("yT", [C, m_tok], BF16, kind="ExternalInput").ap()
    wxT = nc.dram_tensor("wxT", [C, XCH], BF16, kind="ExternalInput").ap()
    wyT = nc.dram_tensor("wyT", [C, YCH], BF16, kind="ExternalInput").ap()
    wp = nc.dram_tensor("wp", [HPC * HD, C], BF16, kind="ExternalInput").ap()
    # rotary/norm coeff tiles: [128, 4*TTX*32]: blocks cwe|swo|swe|cwo,
    # each [128, TTX, 32] (token tile-major)
    cq = nc.dram_tensor("cq", [128, 4 * TTX * 32], F32, kind="ExternalInput").ap()
    ck = nc.dram_tensor("ck", [128, 4 * TTX * 32], F32, kind="ExternalInput").ap()
    kw = nc.dram_tensor("kw", [128, HD], F32, kind="ExternalInput").ap()
    out_d = nc.dram_tensor("out", [n_tok, C], F32, kind="ExternalOutput").ap()
    dbg = {}
    if DEBUG:
        for nm, shape in [("d_qkvx", [128, TTX * XCH]), ("d_kvy", [128, TTY * YCH]),
                          ("d_qn", [128, TTX * 2 * HD]), ("d_kxn", [128, TTX * 2 * HD]),
                          ("d_kyn", [128, TTY * 2 * HD]), ("d_qT", [128, n_tok]),
                          ("d_kT", [128, n_tok + m_tok]), ("d_vaug", [128, (TTX + TTY) * 2 * 65]),
                          ("d_oT", [128, n_tok]), ("d_rstdx", [128, TTX * 4]),
                          ("d_p0", [128, 3 * (n_tok // max(1, n_tok // 512))])]:
            dbg[nm] = nc.dram_tensor(nm, shape, F32, kind="ExternalOutput").ap()

    with tile.TileContext(nc) as tc:
        _emit(tc, nc, locals())
    nc.compile()
    return nc


def _emit(tc, nc, g):
    dbg = g["dbg"]
    n_tok, m_tok = g["n_tok"], g["m_tok"]
    TTX, TTY, KC, QB, QW = g["TTX"], g["TTY"], g["KC"], g["QB"], g["QW"]
    XCH, YCH, KCk = g["XCH"], g["YCH"], g["KCk"]
    xT_d, yT_d, wxT_d, wyT_d, wp_d = g["xT"], g["yT"], g["wxT"], g["wyT"], g["wp"]
    cq_d, ck_d, kw_d, out_d = g["cq"], g["ck"], g["kw"], g["out_d"]

    ctx_pools = []

    const = tc.alloc_tile_pool(name="const", bufs=1)
    data = tc.alloc_tile_pool(name="data", bufs=1)
    wide = tc.alloc_tile_pool(name="wide", bufs=1)
    attn_sb = tc.alloc_tile_pool(name="attn", bufs=1)
    work = tc.alloc_tile_pool(name="work", bufs=3)

    # ---- constants ----
    ident = const.tile([128, 128], BF16)
    make_identity(nc, ident[:])
    eps_t = const.tile([128, 1], F32)
    nc.gpsimd.memset(eps_t[:], EPS)

    # ---- load weights first (x-proj starts as soon as chunk 0 lands) ----
    wx_t = [const.tile([128, XCH], BF16, tag=f"wx{i}", name=f"wx{i}") for i in range(KCk)]
    wy_t = [const.tile([128, YCH], BF16, tag=f"wy{i}", name=f"wy{i}") for i in range(KCk)]
    for k in range(KCk):
        nc.sync.dma_start(wx_t[k][:], wxT_d[k * 128:(k + 1) * 128, :])
    wp_t = const.tile([HPC * HD, C], BF16)
    cq_t = const.tile([128, 4 * TTX * 32], F32)
    ck_t = const.tile([128, 4 * TTX * 32], F32)
    kw_t = const.tile([128, HD], F32)

    # ---- wide natural-layout qkv buffers (fp32) ----
    qkvx = wide.tile([128, TTX * XCH], F32)   # [128, t, 384]
    kvy = wide.tile([128, TTY * YCH], F32)    # [128, t, 256]

    # attention operand tiles
    qT = attn_sb.tile([128, n_tok], BF16)          # 2 heads stacked [64h..]
    kT = attn_sb.tile([128, n_tok + m_tok], BF16)  # x cols then y cols
    vaug = attn_sb.tile([128, KC * 2 * 65], BF16)  # [128, kc, h, 65]
    oT = attn_sb.tile([128, n_tok], BF16)          # normalized oT stack
    qn = attn_sb.tile([128, TTX * 2 * HD], BF16)   # natural normed q [128,t,128]
    kxn = attn_sb.tile([128, TTX * 2 * HD], BF16)
    kyn = attn_sb.tile([128, TTY * 2 * HD], BF16)

    # ones column of v_aug
    nc.gpsimd.memset(vaug[:].rearrange("p (kc h c) -> p (kc h) c", h=2, c=65)[:, :, 64:65], 1.0)

    # ================= Phases 1-3 (x side, then y side, pipelined) =====
    psA = tc.alloc_tile_pool(name="psA", bufs=1, space="PSUM")
    psB = tc.alloc_tile_pool(name="psB", bufs=4, space="PSUM")

    qk3f = qkvx[:].rearrange("p (t c) -> p t c", c=XCH)
    kv3f = kvy[:].rearrange("p (t c) -> p t c", c=YCH)
    sqx = wide.tile([128, TTX * 4 * HD], BF16)  # q,kx squares
    sqy = wide.tile([128, TTY * 2 * HD], BF16)
    ssx = work.tile([128, TTX * 4], F32, tag="ssx", bufs=1)
    rmsx = work.tile([128, TTX * 4], F32, tag="rmsx", bufs=1)
    rstdx = work.tile([128, TTX * 4], F32, tag="rstdx", bufs=1)
    ssy = work.tile([128, TTY * 2], F32, tag="ssy", bufs=1)
    rmsy = work.tile([128, TTY * 2], F32, tag="rmsy", bufs=1)
    rstdy = work.tile([128, TTY * 2], F32, tag="rstdy", bufs=1)
    va4 = vaug[:].rearrange("p (kc h c) -> p kc h c", h=2, c=65)
    qk4 = qkvx[:].rearrange("p (t g c) -> p t g c", g=XCH // HD, c=HD)
    kv4 = kvy[:].rearrange("p (t g c) -> p t g c", g=YCH // HD, c=HD)

    # ---- x projection (k-outer over t-blocks so MMs start on first chunk) ----
    xt_tiles = []
    for k in range(KCk):
        t = data.tile([128, n_tok], BF16, tag=f"xy{k}")
        nc.sync.dma_start(t[:], xT_d[k * 128:(k + 1) * 128, :])
        xt_tiles.append(t)
    for k in range(KCk):
        nc.sync.dma_start(wy_t[k][:], wyT_d[k * 128:(k + 1) * 128, :])
    nc.sync.dma_start(cq_t[:], cq_d[:])
    nc.sync.dma_start(ck_t[:], ck_d[:])
    nc.sync.dma_start(kw_t[:], kw_d[:])
    TB = 4
    for tb in range(0, TTX, TB):
        pss = [psA.tile([128, XCH], F32, tag=f"pj{i}", name=f"pj{i}") for i in range(TB)]
        for k in range(KCk):
            for i in range(TB):
                t = tb + i
                nc.tensor.matmul(pss[i][:], xt_tiles[k][:, t * 128:(t + 1) * 128],
                                 wx_t[k][:], start=(k == 0), stop=(k == KCk - 1))
        for i in range(TB):
            t = tb + i
            nc.scalar.copy(qkvx[:, t * XCH:(t + 1) * XCH], pss[i][:])

    yt_tiles = []
    for k in range(KCk):
        t = data.tile([128, m_tok], BF16, tag=f"xy{k}", name=f"yt{k}")
        nc.sync.dma_start(t[:], yT_d[k * 128:(k + 1) * 128, :])
        yt_tiles.append(t)
    nc.sync.dma_start(wp_t[:], wp_d[:])

    # ---- y projection ----
    for tb in range(0, TTY, TB):
        pss = [psA.tile([128, YCH], F32, tag=f"pj{i}", name=f"pjy{i}") for i in range(TB)]
        for k in range(KCk):
            for i in range(TB):
                t = tb + i
                nc.tensor.matmul(pss[i][:], yt_tiles[k][:, t * 128:(t + 1) * 128],
                                 wy_t[k][:], start=(k == 0), stop=(k == KCk - 1))
        for i in range(TB):
            t = tb + i
            nc.scalar.copy(kvy[:, t * YCH:(t + 1) * YCH], pss[i][:])


    # rotary+norm for a tile range [t0, t1). Rotation is linear, so the
    # rstd scale is applied once after rotating raw te/to.
    def rot(entity, coeff, dst, t0, t1, rstd_t):
        ch0 = entity * 2 * HD
        tw = t1 - t0
        cblk = coeff[:].rearrange("p (b t i) -> p b t i", b=4, i=32)[:, :, t0:t1, :]
        dst3 = dst[:].rearrange("p (t c) -> p t c", c=2 * HD)[:, t0:t1, :]
        qk3s = qk3f[:, t0:t1, :]
        for h in range(HPC):
            rs = rstd_t[:].rearrange("p (t g) -> p t g", g=4)[:, t0:t1, 2 * entity + h: 2 * entity + h + 1]
            te = qk3s[:, :, ch0 + 64 * h: ch0 + 64 * h + 32]
            to = qk3s[:, :, ch0 + 64 * h + 32: ch0 + 64 * h + 64]
            raw = work.tile([128, tw * 64], F32, tag="rraw", bufs=3, name="raw")
            raw3 = raw[:].rearrange("p (t i) -> p t i", i=64)
            m1 = work.tile([128, tw * 32], F32, tag="rtm", bufs=4, name="m1")
            m13 = m1[:].rearrange("p (t i) -> p t i", i=32)
            m2 = work.tile([128, tw * 32], F32, tag="rtm", bufs=4, name="m2")
            m23 = m2[:].rearrange("p (t i) -> p t i", i=32)
            nc.vector.tensor_mul(m13, te, cblk[:, 0])
            nc.vector.tensor_mul(m23, to, cblk[:, 1])
            nc.vector.tensor_sub(raw3[:, :, 0:32], m13, m23)
            nc.vector.tensor_mul(m13, te, cblk[:, 2])
            nc.vector.tensor_mul(m23, to, cblk[:, 3])
            nc.vector.tensor_add(raw3[:, :, 32:64], m13, m23)
            nc.vector.tensor_mul(dst3[:, :, 64 * h: 64 * h + 64], raw3,
                                 rs.broadcast_to((128, tw, 64)))

    # ---- x norm/rotary/transposes in quarters (pipelines with x/y proj) ----
    HALF = max(1, TTX // 4)
    for half in range(TTX // HALF):
        t0, t1 = half * HALF, (half + 1) * HALF
        sl4 = slice(t0 * 4, t1 * 4)
        nc.scalar.activation(
            sqx[:].rearrange("p (t c) -> p t c", c=4 * HD)[:, t0:t1, :],
            qk3f[:, t0:t1, 0:4 * HD], AF.Square)
        nc.vector.reduce_sum(
            ssx[:].rearrange("p (t g) -> p t g", g=4)[:, t0:t1, :],
            sqx[:].rearrange("p (t g c) -> p t g c", g=4, c=HD)[:, t0:t1, :, :],
            axis=mybir.AxisListType.X)
        nc.scalar.activation(rmsx[:, sl4], ssx[:, sl4], AF.Sqrt,
                             scale=1.0 / HD, bias=eps_t[:])
        with nc.allow_low_precision(reason="rstd in bf16 for 2x DVE rotary"):
            nc.vector.reciprocal(rstdx[:, sl4], rmsx[:, sl4])
        rot(0, cq_t, qn, t0, t1, rstdx)
        rot(1, ck_t, kxn, t0, t1, rstdx)
        for t in range(t0, t1):
            pt = psB.tile([128, 128], BF16, tag="tr", name="trq")
            nc.tensor.transpose(pt[:], qn[:, t * 128:(t + 1) * 128], ident[:])
            nc.scalar.copy(qT[:, t * 128:(t + 1) * 128], pt[:])
            pt = psB.tile([128, 128], BF16, tag="tr", name="trk")
            nc.tensor.transpose(pt[:], kxn[:, t * 128:(t + 1) * 128], ident[:])
            nc.scalar.copy(kT[:, t * 128:(t + 1) * 128], pt[:])
    # ---- y norm (no rotary) + transposes, per t-block for pipelining ----
    ky4 = kv4[:, :, 0:2, :]
    kyt = work.tile([128, TTY * 2 * HD], BF16, tag="kyt", bufs=1)
    kyt4 = kyt[:].rearrange("p (t g c) -> p t g c", g=2, c=HD)
    kwb4 = kw_t[:].unsqueeze(1).unsqueeze(1)
    YB = 4
    for tb in range(0, TTY, YB):
        te_ = slice(tb, tb + YB)
        s2 = slice(tb * 2, (tb + YB) * 2)
        nc.scalar.activation(
            sqy[:].rearrange("p (t c) -> p t c", c=2 * HD)[:, te_, :],
            kv3f[:, te_, 0:2 * HD], AF.Square)
        nc.vector.reduce_sum(ssy[:].rearrange("p (t g) -> p t g", g=2)[:, te_, :],
                             sqy[:].rearrange("p (t g c) -> p t g c", g=2, c=HD)[:, te_, :, :],
                             axis=mybir.AxisListType.X)
        nc.scalar.activation(rmsy[:, s2], ssy[:, s2], AF.Sqrt, scale=1.0 / HD, bias=eps_t[:])
        with nc.allow_low_precision(reason="rstd for norm scale"):
            nc.vector.reciprocal(rstdy[:, s2], rmsy[:, s2])
        rsy = rstdy[:].rearrange("p (t g) -> p t g", g=2)[:, te_, :].unsqueeze(3).broadcast_to((128, YB, 2, HD))
        nc.vector.tensor_mul(kyt4[:, te_, :, :], ky4[:, te_, :, :], rsy)
        nc.vector.tensor_mul(kyn[:].rearrange("p (t g c) -> p t g c", g=2, c=HD)[:, te_, :, :],
                             kyt4[:, te_, :, :], kwb4.broadcast_to((128, YB, 2, HD)))
        for t in range(tb, tb + YB):
            pt = psB.tile([128, 128], BF16, tag="tr", name="trky")
            nc.tensor.transpose(pt[:], kyn[:, t * 128:(t + 1) * 128], ident[:])
            nc.scalar.copy(kT[:, n_tok + t * 128:n_tok + (t + 1) * 128], pt[:])

    if DEBUG:
        for nm, src_t in [("d_qkvx", qkvx), ("d_kvy", kvy), ("d_qn", qn),
                          ("d_kxn", kxn), ("d_kyn", kyn), ("d_rstdx", rstdx),
                          ("d_qT", qT), ("d_kT", kT), ("d_vaug", vaug)]:
            tmp = work.tile(list(src_t.shape), F32, tag=f"dbg{nm}", bufs=1, name=f"dbg{nm}")
            nc.vector.tensor_copy(tmp[:], src_t[:])
            nc.sync.dma_start(dbg[nm][:], tmp[:])

    psB.release()
    psA.release()

    # v copies into vaug (deferred: fills engine slack at attention start)
    for t in range(TTX):
        nc.vector.tensor_copy(va4[:, t, :, 0:64], qk4[:, t, 4:6, :])
    for t in range(TTY):
        nc.vector.tensor_copy(va4[:, TTX + t, :, 0:64], kv4[:, t, 2:4, :])

    # ================= Phase 4: attention + fused output projection ====
    psC = tc.alloc_tile_pool(name="psC", bufs=1, space="PSUM")
    BATCH = ATTN_BATCH
    batches = [list(range(b, min(b + BATCH, KC))) for b in range(0, KC, BATCH)]

    def emit_po(t, half):
        po = psC.tile([128, 512], F32, tag="po", bufs=2, name="po")
        nc.tensor.matmul(po[:], oT[:, t * 128:(t + 1) * 128],
                         wp_t[:, half * 512:(half + 1) * 512],
                         start=True, stop=True)
        ob = work.tile([128, 512], F32, tag="ob", bufs=3, name="ob")
        nc.vector.tensor_copy(ob[:], po[:])
        nc.sync.dma_start(out_d[t * 128:(t + 1) * 128, half * 512:(half + 1) * 512],
                          ob[:])

    pending = []
    for qb in range(QB):
        o_ps = [psC.tile([65, QW], F32, tag=f"o{h}", name=f"ops{h}") for h in range(2)]
        for batch in batches:
            for _ in range(min(2, len(pending))):
                emit_po(*pending.pop(0))
            for h in range(2):
                sc = psC.tile([128, BATCH * QW], F32, tag=f"sc{h}")
                for j, kc in enumerate(batch):
                    nc.tensor.matmul(sc[:, j * QW:(j + 1) * QW],
                                     kT[64 * h:64 * h + 64, kc * 128:(kc + 1) * 128],
                                     qT[64 * h:64 * h + 64, qb * QW:(qb + 1) * QW],
                                     start=True, stop=True,
                                     tile_position=(64 * h, 0))
                pt = work.tile([128, BATCH * QW], BF16, tag=f"pt{h}", bufs=2)
                w = len(batch) * QW
                nc.scalar.activation(pt[:, :w], sc[:, :w], AF.Exp, scale=SCALE)
                if DEBUG and qb == 0 and batch[0] == 0:
                    tmp = work.tile([128, BATCH * QW], F32, tag="dbgpt", bufs=2, name=f"dbgpt{h}")
                    nc.vector.tensor_copy(tmp[:], pt[:])
                    if h == 0:
                        nc.sync.dma_start(dbg["d_p0"][:], tmp[:])
                for j, kc in enumerate(batch):
                    nc.tensor.matmul(o_ps[h][:],
                                     va4[:, kc, h, :],
                                     pt[:, j * QW:(j + 1) * QW],
                                     start=(kc == 0), stop=(kc == KC - 1),
                                     skip_group_check=True)
        for h in range(2):
            osb = work.tile([65, QW], F32, tag="osb", bufs=2, name="osb")
            nc.vector.tensor_copy(osb[:], o_ps[h][:])  # frees the o bank fast
            zr = work.tile([1, QW], F32, tag="zr", bufs=2)
            nc.vector.reciprocal(zr[:], osb[64:65, :])
            zb = work.tile([64, QW], F32, tag="zb", bufs=2)
            nc.gpsimd.partition_broadcast(zb[:], zr[:])
            nc.vector.tensor_mul(oT[64 * h:64 * h + 64, qb * QW:(qb + 1) * QW],
                                 osb[0:64, :], zb[:])
        pending += [(t, half) for t in range(qb * (QW // 128), (qb + 1) * (QW // 128))
                    for half in range(C // 512)]
    for t, half in pending:
        emit_po(t, half)
    psC.release()

    if DEBUG:
        tmp = work.tile([128, n_tok], F32, tag="dbgoT", bufs=1, name="dbgoT")
        nc.vector.tensor_copy(tmp[:], oT[:])
        nc.sync.dma_start(dbg["d_oT"][:], tmp[:])

    for p in (work, attn_sb, wide, data, const):
        p.release()


# ---------------- host side ----------------

_PERM = np.concatenate([np.arange(0, HD, 2), np.arange(1, HD, 2)])  # evens, odds


def make_in_maps(x, y, pos, w_qkv_x, w_kv_y, w_proj, q_norm_w, k_norm_w,
                 n_tok, m_tok, ncores=NCORES):
    bf = ml_dtypes.bfloat16
    x2 = np.ascontiguousarray(x.reshape(n_tok, C).T).astype(bf)   # [C, n]
    y2 = np.ascontiguousarray(y.reshape(m_tok, C).T).astype(bf)
    cos = pos[:, :, 0].astype(np.float32)  # [n_tok, 32]
    sin = pos[:, :, 1].astype(np.float32)
    TTX = n_tok // 128

    def coeff_tiles(w):
        we = w[_PERM][:HD // 2].astype(np.float32)  # weights for even slots
        wo = w[_PERM][HD // 2:].astype(np.float32)
        blocks = [cos * we, sin * wo, sin * we, cos * wo]  # cwe swo swe cwo
        # each [n_tok, 32] -> [128, TTX, 32] with token t = tile*128 + p
        arr = np.stack([b.reshape(TTX, 128, 32).transpose(1, 0, 2) for b in blocks])
        return np.ascontiguousarray(arr.transpose(1, 0, 2, 3).reshape(128, 4 * TTX * 32))

    cq = coeff_tiles(q_norm_w)
    ck = coeff_tiles(k_norm_w)
    kw = np.broadcast_to(k_norm_w[_PERM].astype(np.float32), (128, HD)).copy()

    in_maps = []
    for c in range(ncores):
        heads = [HPC * c + i for i in range(HPC)]
        q_rows = np.concatenate([h * HD + _PERM for h in heads])
        kx_rows = np.concatenate([C + h * HD + _PERM for h in heads])
        vx_rows = np.concatenate([2 * C + h * HD + np.arange(HD) for h in heads])
        wx = w_qkv_x[np.concatenate([q_rows, kx_rows, vx_rows])]  # [384, C]
        ky_rows = np.concatenate([h * HD + _PERM for h in heads])
        vy_rows = np.concatenate([C + h * HD + np.arange(HD) for h in heads])
        wy = w_kv_y[np.concatenate([ky_rows, vy_rows])]  # [256, C]
        wpc = w_proj[:, heads[0] * HD:(heads[-1] + 1) * HD].T  # [128, C]
        in_maps.append({
            "xT": x2, "yT": y2,
            "wxT": np.ascontiguousarray(wx.T).astype(bf),
            "wyT": np.ascontiguousarray(wy.T).astype(bf),
            "wp": np.ascontiguousarray(wpc).astype(bf),
            "cq": cq, "ck": ck, "kw": kw,
        })
    return in_maps


_CACHE = {}


def _get_nc(n_tok, m_tok):
    key = (n_tok, m_tok)
    if key not in _CACHE:
        _CACHE[key] = build_nc(n_tok, m_tok)
    return _CACHE[key]


def run(x, y, pos, w_qkv_x, w_kv_y, w_proj, b_proj, q_norm_w, k_norm_w, **kw):
    B, n_tok, _ = x.shape
    m_tok = y.shape[1]
    nc = _get_nc(n_tok, m_tok)
    in_maps = make_in_maps(np.asarray(x), np.asarray(y), np.asarray(pos),
                           np.asarray(w_qkv_x), np.asarray(w_kv_y),
                           np.asarray(w_proj), np.asarray(q_norm_w),
                           np.asarray(k_norm_w), n_tok, m_tok)
    res = run_bass_kernel_spmd(nc, in_maps, core_ids=list(range(NCORES)), **kw)
    acc = np.zeros((n_tok, C), np.float64)
    for r in res.results:
        acc += r["out"].astype(np.float64)
    out = (acc + np.asarray(b_proj)[None, :].astype(np.float64)).astype(np.float32)
    return out.reshape(B, n_tok, C), res


def kernel(x, y, pos, w_qkv_x, w_kv_y, w_proj, b_proj, q_norm_w, k_norm_w):
    out, _ = run(x, y, pos, w_qkv_x, w_kv_y, w_proj, b_proj, q_norm_w, k_norm_w)
    return out

